# revision 39
# baseline (speedup 1.0000x reference)
"""AutoQuantConv2d Trainium2 kernel.

Computes conv2d(fake_quant_nvfp4(x), fake_quant_nvfp4(w)) for
x [32,256,64,64] f32, w [256,256,3,3] f32, stride 1, pad 1, NCHW/OIHW.

Sharding: data-parallel over batch — each of the 8 NeuronCores gets 4
images and the full weight; outputs are concatenated on host.

On-core pipeline (v5):
  1. NVFP4 fake-quant, exact fp32 bit arithmetic (no division):
       amax   = blocked absmax (16 contiguous elements)        [DVE reduce]
       q      = (v + t) - t,  t = max(v & 0x7f800000, scale) * 3*2^21
     as ONE custom fused DVE op.  q is E2M1 x pow2 — exact in fp8e5,
     so the matmul runs fp8 DoubleRow with no prescale/descale.
  2. The DVE is the preamble's serial resource (~1.3 us per op incl.
     ~0.7 us fixed overhead), so quant ops are few and large, emitted
     in exactly the order the consumers need them: w-oc0 ic-halves,
     image-0 row strips (17/16/16/15 rows x both chunks in one op),
     w-oc1 halves interleaved between strips, then images 1-3.
  3. ~12 dummy DoubleRow matmuls on a small zeroed tile warm the PE
     HAM clock gate (4/8 -> 8/8) during the DMA wait so transposes and
     the conv sweep run at 2.4 GHz.
  4. Weight transposes are plain matmuls against identity (N=128,
     ~85 ns warm), packed 4-per-PSUM-bank, one ScalarE cast-evac per
     bank into a packed lhsT tile [ic, 2(chunk), 9(tap), 128(oc)] fp8.
  5. conv2d as implicit GEMM: 576 DoubleRow matmuls [K=256, M=128,
     N=512] at the ~218 ns warm issue floor.  Image 0 oc0 runs in
     2-bank groups chasing the strip quants; everything else in
     4-bank quads.  One PSUM pool, 8 banks.
  6. ScalarE evacuates PSUM -> one [128, 2048] SBUF tile per quad;
     ONE output DMA per quad on the GpSimd queue.  The final group is
     bank-major (9 taps per bank then evac+DMA) so only one bank's
     evac + 256 KB DMA remain after the last matmul.
"""

import numpy as np

import concourse.bass as bass
import concourse.mybir as mybir
from concourse.tile import TileContext
from concourse.bass_utils import run_bass_kernel_spmd
from concourse import masks
from contextlib import ExitStack

AO = mybir.AluOpType
F32 = mybir.dt.float32
I32 = mybir.dt.int32
BF16 = mybir.dt.bfloat16
FP8 = mybir.dt.float8e5
DR = mybir.MatmulPerfMode.DoubleRow
COPY = mybir.ActivationFunctionType.Copy

N_CORES = 8
N_PER = 4          # images per core
C = 256            # input channels
O = 256            # output channels
H = W = 64
HP = WP = 66       # padded spatial
F = H * W          # 4096 pixels per channel
NB = F // 16       # 256 quant blocks per channel row
KF = C * 9         # 2304 flattened weight row per output channel
FPLANE = 4368      # 66*66 padded to a multiple of 16

MASK_EXP = 0x7F800000
K_MAGIC = 6291456.0  # 3 * 2^21: t = floor_pow2(max(|v|,scale)) * K rounds to grid

# image-0 row strips (quant rows); the first two tiny strips unlock
# 1-bank conv groups over h-blocks 0 and 1, the rest 2-bank groups
ROW_SPLITS = [(0, 9), (9, 17), (17, 33), (33, 49), (49, 64)]
# images 1-3 h-halves (aligned so hq0 quads need only the first half)
IMG_SPLITS = [(0, 34), (34, 64)]
N_DUMMY = 12       # PE warm-up matmuls during the preamble DMA wait


# ---------------------------------------------------------------------------
# custom fused DVE op: q = (v + t) - t, t = max(v & expmask, scale) * K
# ---------------------------------------------------------------------------
def _get_fused_quant_op():
    from concourse.dve_ops import OPS, DveOp
    import concourse.dve_ops as dm
    from concourse.dve_spec import (
        Spec, Src0, Src1, Bin, lower, maxx, _has_src1, C0, C1, C2,
    )
    from concourse.dve_uop import DveOpSpec, AluOp

    name = "ANT_NVFP4_FUSED3"
    for op in OPS:
        if op.name == name:
            return op
    # t = max(floor_pow2(|v|), floor_pow2(amax*2/6)) * K, the whole scale
    # computation in-op (C2 = 2/6 via imm2).  floor_pow2(2x) == 2*floor_pow2(x)
    # folds the +1<<23 doubling into the prescale; no eps clamp is needed:
    # an all-zero block gives scale=0 -> t=0 -> q=v=0 exactly.
    # operand roles: Src0 = broadcast amax (3D ok on the primary port),
    # Src1 = the flat activation data (1 free dim, required by the imm2 struct)
    t = Bin(
        AluOp.MULTIPLY,
        maxx(
            Bin(AluOp.BITWISE_AND, Src1, C0),
            Bin(AluOp.BITWISE_AND, Bin(AluOp.MULTIPLY, Src0, C2), C0),
        ),
        C1,
    )
    spec = Spec(
        body=Bin(AluOp.SUBTRACT, Bin(AluOp.ADD, Src1, t), t),
        reference=lambda in0, in1, s0, s1, imm2: in1,
    )
    shas = {}
    for ver in ("v3", "v4"):
        uops = lower(spec, ver=ver)
        shas[ver] = DveOpSpec(name=name, uops=uops, rd1_en=_has_src1(spec)).sha(ver)
    op = DveOp(name, spec, False, uops_sha=shas)
    OPS.append(op)
    dm._SUB_OPCODE_FOR_NAME[name] = dm._CUSTOM_DVE_ROW_BASE + len(OPS) - 1
    return op


def _split_waits(nc, maxw=1):
    """walrus here rejects >1 sync-wait per instruction; hoist extras onto
    preceding same-engine NOPs."""
    bbs = []
    for fn in nc.m.functions:
        for bb in fn.blocks:
            bbs.append((bb, list(bb.instructions)))
    new_lists = []
    for bb, insts in bbs:
        out = []
        for inst in insts:
            si = inst.sync_info
            waits = list(si.on_wait) if si and si.on_wait else []
            if len(waits) > maxw:
                chunks = [waits[i : i + maxw] for i in range(0, len(waits), maxw)]
                eng = nc.engines[inst.engine]
                for chunk in chunks[:-1]:
                    bi = eng.nop(nofuse=True)
                    ni = bi.ins if hasattr(bi, "ins") else bi
                    ni.sync_info = mybir.SyncInfo(on_wait=chunk, on_update=[])
                    out.append(ni)
                inst.sync_info = mybir.SyncInfo(
                    on_wait=chunks[-1], on_update=list(si.on_update or [])
                )
            out.append(inst)
        new_lists.append((bb, out))
    for bb, out in new_lists:
        bb.instructions = out


def _emit_quant(nc, qop, maskt, xd_ap, nblocks, amax, out_ap):
    """NVFP4 fake-quant of SBUF AP xd_ap ([128, nblocks*16] f32 when
    flattened) into out_ap."""
    nc.vector.tensor_reduce(
        amax[:, :],
        xd_ap.rearrange("p (b s) -> p b s", s=16),
        axis=mybir.AxisListType.X,
        op=AO.max,
        apply_absolute_value=True,
    )
    # the fused op computes the pow2 scale from the raw amax in-op
    nc.vector._custom_dve(
        qop,
        out=out_ap,
        in0=amax[:, :].broadcast_to([128, nblocks, 16]),
        in1=xd_ap,
        s0=maskt[:, :],
        s1=K_MAGIC,
        imm2=2.0 / 6.0,
    )


def _build():
    qop = _get_fused_quant_op()
    nc = bass.Bass(trn_type="TRN2")
    x = nc.dram_tensor("x", [N_PER, C, H, W], F32, kind="ExternalInput")
    w = nc.dram_tensor("w", [O, C, 3, 3], F32, kind="ExternalInput")
    out = nc.dram_tensor("out", [N_PER, O, H, W], F32, kind="ExternalOutput")

    with TileContext(nc) as tc:
        with ExitStack() as ctx:
            wpool = ctx.enter_context(tc.tile_pool(name="wpool", bufs=1))
            lpool = ctx.enter_context(tc.tile_pool(name="lpool", bufs=1))
            xqpool = ctx.enter_context(tc.tile_pool(name="xqpool", bufs=1))
            xdpool = ctx.enter_context(tc.tile_pool(name="xdpool", bufs=4))
            smpool = ctx.enter_context(tc.tile_pool(name="smpool", bufs=3))
            obpool = ctx.enter_context(tc.tile_pool(name="obpool", bufs=3))
            pspool = ctx.enter_context(tc.tile_pool(name="ps", bufs=7, space="PSUM"))
            dpool = ctx.enter_context(tc.tile_pool(name="dp", bufs=1, space="PSUM"))

            # ---- init constants (GpSimd) ----
            maskt = wpool.tile([128, 1], F32, name="maskt", tag="maskt")
            nc.gpsimd.memset(maskt[:, :].bitcast(I32), MASK_EXP)
            ident = wpool.tile([128, 128], BF16, name="ident", tag="ident")
            masks.make_identity(nc, ident[:, :])
            # small zero tile feeding the PE warm-up matmuls
            ztile = wpool.tile([128, 1280], FP8, name="ztile", tag="ztile")
            nc.gpsimd.memset(ztile[:, :], 0.0)

            # ---- persistent per-image padded fp8 tiles; zero borders ----
            xq_tiles = []
            for n in range(N_PER):
                t = xqpool.tile([128, 2, FPLANE], FP8, name=f"xq{n}", tag=f"xq{n}")
                tv = t[:, :, 0 : HP * WP].rearrange("p c (h w) -> p c h w", h=HP)
                nc.gpsimd.memset(tv[:, :, 0, :], 0.0)
                nc.gpsimd.memset(tv[:, :, HP - 1, :], 0.0)
                nc.gpsimd.memset(tv[:, :, 1 : HP - 1, 0], 0.0)
                nc.gpsimd.memset(tv[:, :, 1 : HP - 1, WP - 1], 0.0)
                xq_tiles.append(t)

            # ---- PE warm-up: dummy DoubleRow matmuls on zeroed SBUF ----
            # HAM un-throttles (1.2 -> 2.4 GHz) after ~3.4 us of sustained
            # PE activity; these run during the input-DMA wait so the
            # transposes and the conv sweep start warm.
            dumm = dpool.tile([128, 512], F32, name="dummy", tag="dummy")
            dlhs = ztile[:, 0:256].rearrange("p (c m) -> p c m", c=2)
            drhs = ztile[:, 256:1280].rearrange("p (c n) -> p c n", c=2)
            for i in range(N_DUMMY):
                nc.tensor.matmul(
                    dumm[:, :], dlhs, drhs, start=True, stop=True, perf_mode=DR
                )

            # ---- input DMAs (Sync queue), in priority order ----
            wf = {}
            for oc in range(2):
                wf[oc] = wpool.tile([128, KF], F32, name=f"wf{oc}", tag=f"wf{oc}")

            def dma_w_half(oc, half):
                sl = slice(half * 1152, (half + 1) * 1152)
                nc.sync.dma_start(
                    out=wf[oc][:, sl],
                    in_=w[oc * 128 : (oc + 1) * 128, :, :, :].rearrange(
                        "o i kh kw -> o (i kh kw)"
                    )[:, sl],
                )

            def dma_strip(si):
                r0, r1 = ROW_SPLITS[si]
                a = xdpool.tile(
                    [128, 2, (r1 - r0) * W], F32, name=f"xd0_{si}", tag="xd"
                )
                nc.sync.dma_start(
                    out=a[:, :, :],
                    in_=x[0, :, r0:r1, :].rearrange("(c2 c) h w -> c c2 (h w)", c2=2),
                )
                return a

            dma_w_half(0, 0)
            dma_w_half(0, 1)
            # both oc-halves of w before the strips: the DMA queue is FIFO,
            # and the whole weight path (quant + transposes) now runs at the
            # fast preamble rate before the conv sweep begins
            dma_w_half(1, 0)
            dma_w_half(1, 1)
            # image-0 row strips, both 128-channel chunks in one DMA
            xd0 = {}
            for si in range(5):
                xd0[si] = dma_strip(si)
            # remaining images in h-halves split at row 34 so the hq0 quads
            # (which read quant rows 0..32) depend only on the first half
            xds = {}
            for n in range(1, N_PER):
                for hi, (r0, r1) in enumerate(IMG_SPLITS):
                    xd = xdpool.tile(
                        [128, 2, (r1 - r0) * W], F32, name=f"xd_{n}_{hi}", tag="xd"
                    )
                    nc.sync.dma_start(
                        out=xd[:, :, :],
                        in_=x[n, :, r0:r1, :].rearrange(
                            "(c2 c) h w -> c c2 (h w)", c2=2
                        ),
                    )
                    xds[(n, hi)] = xd

            # ---- weight quant (DVE, per ic-half) + matmul transposes ----
            # packed lhsT per oc half: [ic, c_chunk, tap, oc_mod] fp8
            lhsT = {}
            for oc in range(2):
                lhsT[oc] = lpool.tile(
                    [128, 2, 9, 128], FP8, name=f"lhsT{oc}", tag=f"lhsT{oc}"
                )
            wqd = {}

            def emit_w_quant_half(oc, half):
                if oc not in wqd:
                    wqd[oc] = wpool.tile(
                        [128, KF], BF16, name=f"wqd{oc}", tag=f"wqd{oc}"
                    )
                sl = slice(half * 1152, (half + 1) * 1152)
                wam = wpool.tile(
                    [128, 72], F32, name=f"wam{oc}_{half}", tag=f"wam{oc}", bufs=2
                )
                _emit_quant(nc, qop, maskt, wf[oc][:, sl], 72, wam, wqd[oc][:, sl])

            def emit_w_transposes(oc, c):
                # transpose = plain matmul vs identity (N=128, warm ~85 ns);
                # 4 taps packed per PSUM bank, one ScalarE cast-evac per bank
                wv = wqd[oc][:, :].rearrange("p (i k) -> p k i", k=9)
                for t0, ntap in ((0, 4), (4, 4), (8, 1)):
                    wt = pspool.tile(
                        [128, 4, 128], F32, name=f"wt{oc}{c}{t0}", tag="ps"
                    )
                    for j in range(ntap):
                        nc.tensor.matmul(
                            wt[:, j, :],
                            wv[:, t0 + j, c * 128 : (c + 1) * 128],
                            ident[:, :],
                            start=True,
                            stop=True,
                        )
                    nc.scalar.activation(
                        lhsT[oc][:, c, t0 : t0 + ntap, :],
                        wt[:, 0:ntap, :],
                        COPY,
                        scale=1.0,
                    )

            def emit_strip_quant(si):
                # one combined-chunk reduce, then one quant per chunk (the
                # custom DVE op's output AP is capped at 2 free dims)
                r0, r1 = ROW_SPLITS[si]
                nb2 = (r1 - r0) * 4  # blocks per chunk
                am = smpool.tile(
                    [128, 2 * nb2], F32, name=f"am0_{si}", tag=f"am0_{si}", bufs=1
                )
                nc.vector.tensor_reduce(
                    am[:, :],
                    xd0[si][:, :, :].rearrange("p c (b s) -> p (c b) s", s=16),
                    axis=mybir.AxisListType.X,
                    op=AO.max,
                    apply_absolute_value=True,
                )
                for c in range(2):
                    nc.vector._custom_dve(
                        qop,
                        out=xq0v[:, c, 1 + r0 : 1 + r1, 1 : W + 1],
                        in0=am[:, c * nb2 : (c + 1) * nb2].broadcast_to(
                            [128, nb2, 16]
                        ),
                        in1=xd0[si][:, c, :],
                        s0=maskt[:, :],
                        s1=K_MAGIC,
                        imm2=2.0 / 6.0,
                    )

            xq0v = xq_tiles[0][:, :, 0 : HP * WP].rearrange("p c (h w) -> p c h w", h=HP)

            def fillers(k):
                # warm-keeping dummy matmuls between real PE work items:
                # they run during DVE/DMA waits so HAM never re-throttles
                for i in range(k):
                    nc.tensor.matmul(
                        dumm[:, :], dlhs, drhs, start=True, stop=True,
                        perf_mode=DR,
                    )

            # DVE emission order == need order.  The WHOLE weight path
            # (both oc halves) is quantized and transposed before strip 0:
            # it runs at the fast preamble DVE rate while the PE is warming
            # up, and the conv sweep then never waits on weights.
            emit_w_quant_half(0, 0)
            emit_w_quant_half(0, 1)
            emit_w_transposes(0, 0)
            fillers(6)
            emit_w_transposes(0, 1)
            fillers(8)
            emit_w_quant_half(1, 0)
            emit_w_quant_half(1, 1)
            emit_w_transposes(1, 0)
            fillers(24)
            emit_w_transposes(1, 1)
            emit_strip_quant(0)
            fillers(20)

            # ---- conv groups ----
            def emit_group(n, oc, hbs, xqv):
                """taps outer, banks inner; one packed evac tile + one DMA."""
                pss = [
                    pspool.tile([128, 512], F32, name=f"ps_{n}_{oc}_{hb}", tag="ps")
                    for hb in hbs
                ]
                k = 0
                for kh in range(3):
                    for kw in range(3):
                        for j, hb in enumerate(hbs):
                            rhs = xqv[
                                :, :, hb * 8 + kh : hb * 8 + kh + 8, kw : kw + 64
                            ]
                            nc.tensor.matmul(
                                pss[j][:, :],
                                lhsT[oc][:, :, kh * 3 + kw, :],
                                rhs,
                                start=(k == 0),
                                stop=(k == 8),
                                perf_mode=DR,
                            )
                        k += 1
                nb = len(hbs)
                ob = obpool.tile(
                    [128, nb * 512], F32, name=f"ob_{n}_{oc}_{hbs[0]}", tag="ob"
                )
                for j in range(nb):
                    nc.scalar.activation(
                        ob[:, j * 512 : (j + 1) * 512], pss[j][:, :], COPY, scale=1.0
                    )
                r0 = hbs[0] * 8
                dst = out[n, oc * 128 : (oc + 1) * 128, r0 : r0 + nb * 8, :]
                src = ob[:, :].rearrange("p (h w) -> p h w", h=nb * 8)
                # alternate output queues to halve per-queue drain backlog
                if oc == 0:
                    nc.gpsimd.dma_start(out=dst, in_=src)
                else:
                    nc.scalar.dma_start(out=dst, in_=src)

            def emit_final_piece(n, oc, xqv, r0, nrow, tag_id):
                """one bank-major piece: nrow output rows, 9 taps, evac+DMA"""
                npx = nrow * 64
                ps = pspool.tile([128, 512], F32, name=f"psf_{tag_id}", tag="ps")
                k = 0
                for kh in range(3):
                    for kw in range(3):
                        rhs = xqv[:, :, r0 + kh : r0 + kh + nrow, kw : kw + 64]
                        nc.tensor.matmul(
                            ps[:, 0:npx],
                            lhsT[oc][:, :, kh * 3 + kw, :],
                            rhs,
                            start=(k == 0),
                            stop=(k == 8),
                            perf_mode=DR,
                        )
                        k += 1
                ob = obpool.tile(
                    [128, 512], F32, name=f"obf_{tag_id}", tag="obf", bufs=6
                )
                nc.scalar.activation(ob[:, 0:npx], ps[:, 0:npx], COPY, scale=1.0)
                dst = out[n, oc * 128 : (oc + 1) * 128, r0 : r0 + nrow, :]
                src = ob[:, 0:npx].rearrange("p (h w) -> p h w", h=nrow)
                # the last two pieces drain on the scalar queue right
                # behind their own evacs (the gpsimd drain then ends early)
                if tag_id <= 2:
                    nc.gpsimd.dma_start(out=dst, in_=src)
                else:
                    nc.scalar.dma_start(out=dst, in_=src)

            def emit_group_final(n, oc, xqv, hbs=None):
                """bank-major drain: each bank gets its 9 taps consecutively,
                then evac + its own DMA; the last bank is split into two
                half-banks so only ~1 us of work remains after the last
                matmul."""
                hbs = list(hbs if hbs is not None else range(8))
                for i, hb in enumerate(hbs):
                    if i < len(hbs) - 1:
                        emit_final_piece(n, oc, xqv, hb * 8, 8, i)
                    else:
                        emit_final_piece(n, oc, xqv, hb * 8, 4, i)
                        emit_final_piece(n, oc, xqv, hb * 8 + 4, 4, i + 1)

            def emit_img_quant_half(n, hi):
                """DVE blocked absmax + fake-quant for one image h-half."""
                r0, r1 = IMG_SPLITS[hi]
                nb = (r1 - r0) * 4
                xd = xds[(n, hi)]
                xqv = xq_tiles[n][:, :, 0 : HP * WP].rearrange(
                    "p c (h w) -> p c h w", h=HP
                )
                for c in range(2):
                    am = smpool.tile(
                        [128, nb], F32, name=f"am_{n}_{hi}_{c}", tag="amax", bufs=4
                    )
                    _emit_quant(
                        nc, qop, maskt, xd[:, c, :], nb, am,
                        xqv[:, c, 1 + r0 : 1 + r1, 1 : W + 1],
                    )

            # ---- image 0: 2-bank groups, oc0/oc1 interleaved per strip ----
            # The DVE delivers one quantized strip every ~5 us but a 2-bank
            # group consumes one every ~3.9 us; running BOTH oc halves over
            # each strip doubles the PE work per strip so the DVE keeps up.
            emit_group(0, 0, [0], xq0v)
            emit_group(0, 1, [0], xq0v)
            emit_strip_quant(1)
            emit_group(0, 0, [1], xq0v)
            emit_group(0, 1, [1], xq0v)
            emit_strip_quant(2)
            emit_group(0, 1, [2, 3], xq0v)
            emit_group(0, 0, [2, 3], xq0v)
            emit_strip_quant(3)
            emit_group(0, 0, [4, 5], xq0v)
            emit_group(0, 1, [4, 5], xq0v)
            emit_strip_quant(4)
            emit_group(0, 0, [6, 7], xq0v)
            emit_group(0, 1, [6, 7], xq0v)

            # ---- images 1-3: per-h-half quant gating ----
            # hq0 quads read only quant rows 0..32 (= the first h-half), so
            # the PE order h0-quants / hq0-quads(both oc) / h1-quants /
            # hq1-quads gives the DVE a full 15.7 us of PE work per half and
            # removes the image-transition stall.
            for n in range(1, N_PER):
                xqv = xq_tiles[n][:, :, 0 : HP * WP].rearrange(
                    "p c (h w) -> p c h w", h=HP
                )
                emit_img_quant_half(n, 0)
                emit_group(n, 0, [0, 1, 2, 3], xqv)
                emit_group(n, 1, [0, 1, 2, 3], xqv)
                emit_img_quant_half(n, 1)
                emit_group(n, 0, [4, 5, 6, 7], xqv)
                if n == 3:
                    emit_group_final(3, 1, xqv, hbs=[4, 5, 6, 7])
                else:
                    emit_group(n, 1, [4, 5, 6, 7], xqv)

    mybir.codegen_inst_isa_subclasses(nc)
    _split_waits(nc, maxw=1)
    return nc


_NC_CACHE = None


def _get_nc():
    global _NC_CACHE
    if _NC_CACHE is None:
        _NC_CACHE = _build()
    return _NC_CACHE


def kernel(x: np.ndarray, w: np.ndarray) -> np.ndarray:
    x = np.ascontiguousarray(x, dtype=np.float32)
    w = np.ascontiguousarray(w, dtype=np.float32)
    nc = _get_nc()
    in_maps = [
        {"x": x[i * N_PER : (i + 1) * N_PER], "w": w} for i in range(N_CORES)
    ]
    res = run_bass_kernel_spmd(nc, in_maps, core_ids=list(range(N_CORES)))
    return np.concatenate([res.results[i]["out"] for i in range(N_CORES)], axis=0)


# revision 41
# speedup vs baseline: 1.0023x; 1.0023x over previous
"""AutoQuantConv2d Trainium2 kernel.

Computes conv2d(fake_quant_nvfp4(x), fake_quant_nvfp4(w)) for
x [32,256,64,64] f32, w [256,256,3,3] f32, stride 1, pad 1, NCHW/OIHW.

Sharding: data-parallel over batch — each of the 8 NeuronCores gets 4
images and the full weight; outputs are concatenated on host.

On-core pipeline (v6):
  1. NVFP4 fake-quant, exact fp32 bit arithmetic (no division):
       amax   = blocked absmax (16 contiguous elements)        [DVE reduce]
       q      = (v + t) - t,  t = max(v & 0x7f800000, scale) * 3*2^21
     as ONE custom fused DVE op.  q is E2M1 x pow2 — exact in fp8e5,
     so the matmul runs fp8 DoubleRow with no prescale/descale.
  2. The DVE is the serial bottleneck of the first ~50 us, so quant
     ops are few, large, and emitted in exactly consumer-need order:
     the WHOLE weight path first (both oc halves, at the fast preamble
     DVE rate, so conv never waits on weights), then image-0 in four
     row strips, then images 1-3 in h-halves.
  3. Dummy DoubleRow matmuls on a small zeroed tile run during every
     preamble wait so the PE HAM clock gate stays at 8/8 (2.4 GHz)
     from the first transpose through the whole conv sweep.
  4. Weight transposes are plain matmuls against identity (N=128,
     ~85 ns warm), packed 4-per-PSUM-bank, one ScalarE cast-evac per
     bank into a packed lhsT tile [ic, 2(chunk), 9(tap), 128(oc)] fp8.
  5. conv2d as implicit GEMM: 576 DoubleRow matmuls [K=256, M=128,
     N=512] at the ~218 ns warm issue floor.  Image 0 runs 2-bank
     groups with oc0/oc1 interleaved per strip (the DVE delivers one
     strip per ~5 us; both oc halves consume ~7.9 us of PE per strip).
     Images 1-3 run 4-bank quads gated per h-half: the hq0 quads of
     both oc halves need only the first half's quant, which gives the
     DVE a full 15.7 us of headroom per half and removes every
     image-transition stall.
  6. ScalarE evacuates PSUM -> one [128, 2048] SBUF tile per quad;
     one output DMA per quad (oc0 -> GpSimd queue, oc1 -> ScalarE
     queue).  The final 4 banks drain bank-major with the last bank
     split in half, so only ~1.5 us of evac + DMA trail the last
     matmul before the fixed teardown barrier.
"""

import numpy as np

import concourse.bass as bass
import concourse.mybir as mybir
from concourse.tile import TileContext
from concourse.bass_utils import run_bass_kernel_spmd
from concourse import masks
from contextlib import ExitStack

AO = mybir.AluOpType
F32 = mybir.dt.float32
I32 = mybir.dt.int32
BF16 = mybir.dt.bfloat16
FP8 = mybir.dt.float8e5
DR = mybir.MatmulPerfMode.DoubleRow
COPY = mybir.ActivationFunctionType.Copy

N_CORES = 8
N_PER = 4          # images per core
C = 256            # input channels
O = 256            # output channels
H = W = 64
HP = WP = 66       # padded spatial
F = H * W          # 4096 pixels per channel
NB = F // 16       # 256 quant blocks per channel row
KF = C * 9         # 2304 flattened weight row per output channel
FPLANE = 4368      # 66*66 padded to a multiple of 16

MASK_EXP = 0x7F800000
K_MAGIC = 6291456.0  # 3 * 2^21: t = floor_pow2(max(|v|,scale)) * K rounds to grid

# image-0 row strips (quant rows); strip s unlocks the 2-bank conv
# group over h-blocks (2s, 2s+1)
ROW_SPLITS = [(0, 17), (17, 33), (33, 49), (49, 64)]
# images 1-3 h-halves (aligned so hq0 quads need only the first half)
IMG_SPLITS = [(0, 34), (34, 64)]
N_DUMMY = 12       # PE warm-up matmuls during the preamble DMA wait


# ---------------------------------------------------------------------------
# custom fused DVE op: q = (v + t) - t, t = max(v & expmask, scale) * K
# ---------------------------------------------------------------------------
def _get_fused_quant_op():
    from concourse.dve_ops import OPS, DveOp
    import concourse.dve_ops as dm
    from concourse.dve_spec import (
        Spec, Src0, Src1, Bin, lower, maxx, _has_src1, C0, C1, C2,
    )
    from concourse.dve_uop import DveOpSpec, AluOp

    name = "ANT_NVFP4_FUSED3"
    for op in OPS:
        if op.name == name:
            return op
    # t = max(floor_pow2(|v|), floor_pow2(amax*2/6)) * K, the whole scale
    # computation in-op (C2 = 2/6 via imm2).  floor_pow2(2x) == 2*floor_pow2(x)
    # folds the +1<<23 doubling into the prescale; no eps clamp is needed:
    # an all-zero block gives scale=0 -> t=0 -> q=v=0 exactly.
    # operand roles: Src0 = broadcast amax (3D ok on the primary port),
    # Src1 = the flat activation data (1 free dim, required by the imm2 struct)
    t = Bin(
        AluOp.MULTIPLY,
        maxx(
            Bin(AluOp.BITWISE_AND, Src1, C0),
            Bin(AluOp.BITWISE_AND, Bin(AluOp.MULTIPLY, Src0, C2), C0),
        ),
        C1,
    )
    spec = Spec(
        body=Bin(AluOp.SUBTRACT, Bin(AluOp.ADD, Src1, t), t),
        reference=lambda in0, in1, s0, s1, imm2: in1,
    )
    shas = {}
    for ver in ("v3", "v4"):
        uops = lower(spec, ver=ver)
        shas[ver] = DveOpSpec(name=name, uops=uops, rd1_en=_has_src1(spec)).sha(ver)
    op = DveOp(name, spec, False, uops_sha=shas)
    OPS.append(op)
    dm._SUB_OPCODE_FOR_NAME[name] = dm._CUSTOM_DVE_ROW_BASE + len(OPS) - 1
    return op


def _split_waits(nc, maxw=1):
    """walrus here rejects >1 sync-wait per instruction; hoist extras onto
    preceding same-engine NOPs."""
    bbs = []
    for fn in nc.m.functions:
        for bb in fn.blocks:
            bbs.append((bb, list(bb.instructions)))
    new_lists = []
    for bb, insts in bbs:
        out = []
        for inst in insts:
            si = inst.sync_info
            waits = list(si.on_wait) if si and si.on_wait else []
            if len(waits) > maxw:
                chunks = [waits[i : i + maxw] for i in range(0, len(waits), maxw)]
                eng = nc.engines[inst.engine]
                for chunk in chunks[:-1]:
                    bi = eng.nop(nofuse=True)
                    ni = bi.ins if hasattr(bi, "ins") else bi
                    ni.sync_info = mybir.SyncInfo(on_wait=chunk, on_update=[])
                    out.append(ni)
                inst.sync_info = mybir.SyncInfo(
                    on_wait=chunks[-1], on_update=list(si.on_update or [])
                )
            out.append(inst)
        new_lists.append((bb, out))
    for bb, out in new_lists:
        bb.instructions = out


def _emit_quant(nc, qop, maskt, xd_ap, nblocks, amax, out_ap):
    """NVFP4 fake-quant of SBUF AP xd_ap ([128, nblocks*16] f32 when
    flattened) into out_ap."""
    nc.vector.tensor_reduce(
        amax[:, :],
        xd_ap.rearrange("p (b s) -> p b s", s=16),
        axis=mybir.AxisListType.X,
        op=AO.max,
        apply_absolute_value=True,
    )
    # the fused op computes the pow2 scale from the raw amax in-op
    nc.vector._custom_dve(
        qop,
        out=out_ap,
        in0=amax[:, :].broadcast_to([128, nblocks, 16]),
        in1=xd_ap,
        s0=maskt[:, :],
        s1=K_MAGIC,
        imm2=2.0 / 6.0,
    )


def _build():
    qop = _get_fused_quant_op()
    nc = bass.Bass(trn_type="TRN2")
    x = nc.dram_tensor("x", [N_PER, C, H, W], F32, kind="ExternalInput")
    w = nc.dram_tensor("w", [O, C, 3, 3], F32, kind="ExternalInput")
    out = nc.dram_tensor("out", [N_PER, O, H, W], F32, kind="ExternalOutput")

    with TileContext(nc) as tc:
        with ExitStack() as ctx:
            wpool = ctx.enter_context(tc.tile_pool(name="wpool", bufs=1))
            lpool = ctx.enter_context(tc.tile_pool(name="lpool", bufs=1))
            xqpool = ctx.enter_context(tc.tile_pool(name="xqpool", bufs=1))
            xdpool = ctx.enter_context(tc.tile_pool(name="xdpool", bufs=4))
            smpool = ctx.enter_context(tc.tile_pool(name="smpool", bufs=3))
            obpool = ctx.enter_context(tc.tile_pool(name="obpool", bufs=3))
            pspool = ctx.enter_context(tc.tile_pool(name="ps", bufs=7, space="PSUM"))
            dpool = ctx.enter_context(tc.tile_pool(name="dp", bufs=1, space="PSUM"))

            # ---- init constants (GpSimd) ----
            maskt = wpool.tile([128, 1], F32, name="maskt", tag="maskt")
            nc.gpsimd.memset(maskt[:, :].bitcast(I32), MASK_EXP)
            ident = wpool.tile([128, 128], BF16, name="ident", tag="ident")
            masks.make_identity(nc, ident[:, :])
            # small zero tile feeding the PE warm-up matmuls
            ztile = wpool.tile([128, 1280], FP8, name="ztile", tag="ztile")
            nc.gpsimd.memset(ztile[:, :], 0.0)

            # ---- persistent per-image padded fp8 tiles; zero borders ----
            xq_tiles = []
            for n in range(N_PER):
                t = xqpool.tile([128, 2, FPLANE], FP8, name=f"xq{n}", tag=f"xq{n}")
                tv = t[:, :, 0 : HP * WP].rearrange("p c (h w) -> p c h w", h=HP)
                nc.gpsimd.memset(tv[:, :, 0, :], 0.0)
                nc.gpsimd.memset(tv[:, :, HP - 1, :], 0.0)
                nc.gpsimd.memset(tv[:, :, 1 : HP - 1, 0], 0.0)
                nc.gpsimd.memset(tv[:, :, 1 : HP - 1, WP - 1], 0.0)
                xq_tiles.append(t)

            # ---- PE warm-up: dummy DoubleRow matmuls on zeroed SBUF ----
            # HAM un-throttles (1.2 -> 2.4 GHz) after ~3.4 us of sustained
            # PE activity; these run during the input-DMA wait so the
            # transposes and the conv sweep start warm.
            dumm = dpool.tile([128, 512], F32, name="dummy", tag="dummy")
            dlhs = ztile[:, 0:256].rearrange("p (c m) -> p c m", c=2)
            drhs = ztile[:, 256:1280].rearrange("p (c n) -> p c n", c=2)
            for i in range(N_DUMMY):
                nc.tensor.matmul(
                    dumm[:, :], dlhs, drhs, start=True, stop=True, perf_mode=DR
                )

            # ---- input DMAs (Sync queue), in priority order ----
            wf = {}
            for oc in range(2):
                wf[oc] = wpool.tile([128, KF], F32, name=f"wf{oc}", tag=f"wf{oc}")

            def dma_w_half(oc, half):
                sl = slice(half * 1152, (half + 1) * 1152)
                nc.sync.dma_start(
                    out=wf[oc][:, sl],
                    in_=w[oc * 128 : (oc + 1) * 128, :, :, :].rearrange(
                        "o i kh kw -> o (i kh kw)"
                    )[:, sl],
                )

            def dma_strip(si):
                r0, r1 = ROW_SPLITS[si]
                a = xdpool.tile(
                    [128, 2, (r1 - r0) * W], F32, name=f"xd0_{si}", tag="xd"
                )
                nc.sync.dma_start(
                    out=a[:, :, :],
                    in_=x[0, :, r0:r1, :].rearrange("(c2 c) h w -> c c2 (h w)", c2=2),
                )
                return a

            dma_w_half(0, 0)
            dma_w_half(0, 1)
            # both oc-halves of w before the strips: the DMA queue is FIFO,
            # and the whole weight path (quant + transposes) now runs at the
            # fast preamble rate before the conv sweep begins
            dma_w_half(1, 0)
            dma_w_half(1, 1)
            # image-0 row strips, both 128-channel chunks in one DMA
            xd0 = {}
            for si in range(4):
                xd0[si] = dma_strip(si)
            # remaining images in h-halves split at row 34 so the hq0 quads
            # (which read quant rows 0..32) depend only on the first half
            xds = {}
            for n in range(1, N_PER):
                for hi, (r0, r1) in enumerate(IMG_SPLITS):
                    xd = xdpool.tile(
                        [128, 2, (r1 - r0) * W], F32, name=f"xd_{n}_{hi}", tag="xd"
                    )
                    nc.sync.dma_start(
                        out=xd[:, :, :],
                        in_=x[n, :, r0:r1, :].rearrange(
                            "(c2 c) h w -> c c2 (h w)", c2=2
                        ),
                    )
                    xds[(n, hi)] = xd

            # ---- weight quant (DVE, per ic-half) + matmul transposes ----
            # packed lhsT per oc half: [ic, c_chunk, tap, oc_mod] fp8
            lhsT = {}
            for oc in range(2):
                lhsT[oc] = lpool.tile(
                    [128, 2, 9, 128], FP8, name=f"lhsT{oc}", tag=f"lhsT{oc}"
                )
            wqd = {}

            def emit_w_quant_half(oc, half):
                if oc not in wqd:
                    wqd[oc] = wpool.tile(
                        [128, KF], BF16, name=f"wqd{oc}", tag=f"wqd{oc}"
                    )
                sl = slice(half * 1152, (half + 1) * 1152)
                wam = wpool.tile(
                    [128, 72], F32, name=f"wam{oc}_{half}", tag=f"wam{oc}", bufs=2
                )
                _emit_quant(nc, qop, maskt, wf[oc][:, sl], 72, wam, wqd[oc][:, sl])

            def emit_w_transposes(oc, c):
                # transpose = plain matmul vs identity (N=128, warm ~85 ns);
                # 4 taps packed per PSUM bank, one ScalarE cast-evac per bank
                wv = wqd[oc][:, :].rearrange("p (i k) -> p k i", k=9)
                for t0, ntap in ((0, 4), (4, 4), (8, 1)):
                    wt = pspool.tile(
                        [128, 4, 128], F32, name=f"wt{oc}{c}{t0}", tag="ps"
                    )
                    for j in range(ntap):
                        nc.tensor.matmul(
                            wt[:, j, :],
                            wv[:, t0 + j, c * 128 : (c + 1) * 128],
                            ident[:, :],
                            start=True,
                            stop=True,
                        )
                    nc.scalar.activation(
                        lhsT[oc][:, c, t0 : t0 + ntap, :],
                        wt[:, 0:ntap, :],
                        COPY,
                        scale=1.0,
                    )

            def emit_strip_quant(si):
                # one combined-chunk reduce, then one quant per chunk (the
                # custom DVE op's output AP is capped at 2 free dims)
                r0, r1 = ROW_SPLITS[si]
                nb2 = (r1 - r0) * 4  # blocks per chunk
                am = smpool.tile(
                    [128, 2 * nb2], F32, name=f"am0_{si}", tag=f"am0_{si}", bufs=1
                )
                nc.vector.tensor_reduce(
                    am[:, :],
                    xd0[si][:, :, :].rearrange("p c (b s) -> p (c b) s", s=16),
                    axis=mybir.AxisListType.X,
                    op=AO.max,
                    apply_absolute_value=True,
                )
                for c in range(2):
                    nc.vector._custom_dve(
                        qop,
                        out=xq0v[:, c, 1 + r0 : 1 + r1, 1 : W + 1],
                        in0=am[:, c * nb2 : (c + 1) * nb2].broadcast_to(
                            [128, nb2, 16]
                        ),
                        in1=xd0[si][:, c, :],
                        s0=maskt[:, :],
                        s1=K_MAGIC,
                        imm2=2.0 / 6.0,
                    )

            xq0v = xq_tiles[0][:, :, 0 : HP * WP].rearrange("p c (h w) -> p c h w", h=HP)

            def fillers(k):
                # warm-keeping dummy matmuls between real PE work items:
                # they run during DVE/DMA waits so HAM never re-throttles
                for i in range(k):
                    nc.tensor.matmul(
                        dumm[:, :], dlhs, drhs, start=True, stop=True,
                        perf_mode=DR,
                    )

            # DVE emission order == need order.  The WHOLE weight path
            # (both oc halves) is quantized and transposed before strip 0:
            # it runs at the fast preamble DVE rate while the PE is warming
            # up, and the conv sweep then never waits on weights.
            emit_w_quant_half(0, 0)
            emit_w_quant_half(0, 1)
            emit_w_transposes(0, 0)
            fillers(6)
            emit_w_transposes(0, 1)
            fillers(8)
            emit_w_quant_half(1, 0)
            emit_w_quant_half(1, 1)
            emit_w_transposes(1, 0)
            fillers(24)
            emit_w_transposes(1, 1)
            emit_strip_quant(0)
            fillers(20)

            # ---- conv groups ----
            def emit_group(n, oc, hbs, xqv):
                """taps outer, banks inner; one packed evac tile + one DMA."""
                pss = [
                    pspool.tile([128, 512], F32, name=f"ps_{n}_{oc}_{hb}", tag="ps")
                    for hb in hbs
                ]
                k = 0
                for kh in range(3):
                    for kw in range(3):
                        for j, hb in enumerate(hbs):
                            rhs = xqv[
                                :, :, hb * 8 + kh : hb * 8 + kh + 8, kw : kw + 64
                            ]
                            nc.tensor.matmul(
                                pss[j][:, :],
                                lhsT[oc][:, :, kh * 3 + kw, :],
                                rhs,
                                start=(k == 0),
                                stop=(k == 8),
                                perf_mode=DR,
                            )
                        k += 1
                nb = len(hbs)
                ob = obpool.tile(
                    [128, nb * 512], F32, name=f"ob_{n}_{oc}_{hbs[0]}", tag="ob"
                )
                for j in range(nb):
                    nc.scalar.activation(
                        ob[:, j * 512 : (j + 1) * 512], pss[j][:, :], COPY, scale=1.0
                    )
                r0 = hbs[0] * 8
                dst = out[n, oc * 128 : (oc + 1) * 128, r0 : r0 + nb * 8, :]
                src = ob[:, :].rearrange("p (h w) -> p h w", h=nb * 8)
                # alternate output queues to halve per-queue drain backlog
                if oc == 0:
                    nc.gpsimd.dma_start(out=dst, in_=src)
                else:
                    nc.scalar.dma_start(out=dst, in_=src)

            def emit_final_piece(n, oc, xqv, r0, nrow, tag_id):
                """one bank-major piece: nrow output rows, 9 taps, evac+DMA"""
                npx = nrow * 64
                ps = pspool.tile([128, 512], F32, name=f"psf_{tag_id}", tag="ps")
                k = 0
                for kh in range(3):
                    for kw in range(3):
                        rhs = xqv[:, :, r0 + kh : r0 + kh + nrow, kw : kw + 64]
                        nc.tensor.matmul(
                            ps[:, 0:npx],
                            lhsT[oc][:, :, kh * 3 + kw, :],
                            rhs,
                            start=(k == 0),
                            stop=(k == 8),
                            perf_mode=DR,
                        )
                        k += 1
                ob = obpool.tile(
                    [128, 512], F32, name=f"obf_{tag_id}", tag="obf", bufs=6
                )
                nc.scalar.activation(ob[:, 0:npx], ps[:, 0:npx], COPY, scale=1.0)
                dst = out[n, oc * 128 : (oc + 1) * 128, r0 : r0 + nrow, :]
                src = ob[:, 0:npx].rearrange("p (h w) -> p h w", h=nrow)
                # the last two pieces drain on the scalar queue right
                # behind their own evacs (the gpsimd drain then ends early)
                if tag_id <= 2:
                    nc.gpsimd.dma_start(out=dst, in_=src)
                else:
                    nc.scalar.dma_start(out=dst, in_=src)

            def emit_group_final(n, oc, xqv, hbs=None):
                """bank-major drain: each bank gets its 9 taps consecutively,
                then evac + its own DMA; the last bank is split into two
                half-banks so only ~1 us of work remains after the last
                matmul."""
                hbs = list(hbs if hbs is not None else range(8))
                for i, hb in enumerate(hbs):
                    if i < len(hbs) - 1:
                        emit_final_piece(n, oc, xqv, hb * 8, 8, i)
                    else:
                        emit_final_piece(n, oc, xqv, hb * 8, 4, i)
                        emit_final_piece(n, oc, xqv, hb * 8 + 4, 4, i + 1)

            def emit_img_quant_half(n, hi):
                """DVE blocked absmax + fake-quant for one image h-half."""
                r0, r1 = IMG_SPLITS[hi]
                nb = (r1 - r0) * 4
                xd = xds[(n, hi)]
                xqv = xq_tiles[n][:, :, 0 : HP * WP].rearrange(
                    "p c (h w) -> p c h w", h=HP
                )
                for c in range(2):
                    am = smpool.tile(
                        [128, nb], F32, name=f"am_{n}_{hi}_{c}", tag="amax", bufs=4
                    )
                    _emit_quant(
                        nc, qop, maskt, xd[:, c, :], nb, am,
                        xqv[:, c, 1 + r0 : 1 + r1, 1 : W + 1],
                    )

            # ---- image 0: 2-bank groups, oc0/oc1 interleaved per strip ----
            # The DVE delivers one quantized strip every ~5 us but a 2-bank
            # group consumes one every ~3.9 us; running BOTH oc halves over
            # each strip doubles the PE work per strip so the DVE keeps up.
            emit_group(0, 0, [0, 1], xq0v)
            emit_group(0, 1, [0, 1], xq0v)
            emit_strip_quant(1)
            emit_group(0, 1, [2, 3], xq0v)
            emit_group(0, 0, [2, 3], xq0v)
            emit_strip_quant(2)
            emit_group(0, 0, [4, 5], xq0v)
            emit_group(0, 1, [4, 5], xq0v)
            emit_strip_quant(3)
            emit_group(0, 0, [6, 7], xq0v)
            emit_group(0, 1, [6, 7], xq0v)

            # ---- images 1-3: per-h-half quant gating ----
            # hq0 quads read only quant rows 0..32 (= the first h-half), so
            # the PE order h0-quants / hq0-quads(both oc) / h1-quants /
            # hq1-quads gives the DVE a full 15.7 us of PE work per half and
            # removes the image-transition stall.
            for n in range(1, N_PER):
                xqv = xq_tiles[n][:, :, 0 : HP * WP].rearrange(
                    "p c (h w) -> p c h w", h=HP
                )
                emit_img_quant_half(n, 0)
                emit_group(n, 0, [0, 1, 2, 3], xqv)
                emit_group(n, 1, [0, 1, 2, 3], xqv)
                emit_img_quant_half(n, 1)
                emit_group(n, 0, [4, 5, 6, 7], xqv)
                if n == 3:
                    emit_group_final(3, 1, xqv, hbs=[4, 5, 6, 7])
                else:
                    emit_group(n, 1, [4, 5, 6, 7], xqv)

    mybir.codegen_inst_isa_subclasses(nc)
    _split_waits(nc, maxw=1)
    return nc


_NC_CACHE = None


def _get_nc():
    global _NC_CACHE
    if _NC_CACHE is None:
        _NC_CACHE = _build()
    return _NC_CACHE


def kernel(x: np.ndarray, w: np.ndarray) -> np.ndarray:
    x = np.ascontiguousarray(x, dtype=np.float32)
    w = np.ascontiguousarray(w, dtype=np.float32)
    nc = _get_nc()
    in_maps = [
        {"x": x[i * N_PER : (i + 1) * N_PER], "w": w} for i in range(N_CORES)
    ]
    res = run_bass_kernel_spmd(nc, in_maps, core_ids=list(range(N_CORES)))
    return np.concatenate([res.results[i]["out"] for i in range(N_CORES)], axis=0)
